# revision 15
# baseline (speedup 1.0000x reference)
"""TRN2 Bass kernel for GQA attention (nn_Attention_19533511262498).

Tensor-parallel over heads across 8 NeuronCores: core c owns q-heads
[4c, 4c+4) and kv-head c (wq/wk/wv sharded on the head dim, wo on its
input dim). Each core computes a partial [S, DIM] output (bf16); the
host sums the 8 partials in f32.

All matmuls run bf16 x bf16 (both PE operands must share width class;
bf16 LDWEIGHTS is ~97ns and hides fully under 216ns moving-512
matmuls, unlike f32r's 187ns weight loads). End-to-end max-rel error
vs the f32 reference is ~5e-3 (gate 2e-2).

Phase 1 (projections + RoPE): s-blocks of 512, x streamed as k-quarter
tiles; all 6 output blocks (4 q heads, k, v) accumulate concurrently
in 6 PSUM banks so x tiles die quickly. RoPE via the even/odd row
permutation of wq/wk (host side) -> elementwise ops + a half-tile
partition swap (SBUF-SBUF DMA).

Phase 2 (attention + output projection): causal triangle is exploited
at 128-column granularity: diagonal kv-chunks compute only the valid
column suffix (matmul/exp/sum/pv all sliced), with a per-element
[128,128] triangle mask applied to exp values by the vector engine --
no mask matmuls. Softmax normalizer = ones-column reduce matmul ->
broadcast matmul -> vector reciprocal+mul (no gpsimd). The previous
q-block's output projection matmuls are interleaved INTO the score/pv
chunk stream so the PE never waits on the scalar engine's exp chain.
V is transposed on-chip by 16 PE identity transposes interleaved the
same way.
"""

import ml_dtypes
import numpy as np

import concourse.bacc as bacc
import concourse.tile as tile
from concourse import mybir
from concourse.bass import ts, ds
from concourse.bass_utils import run_bass_kernel_spmd

F32 = mybir.dt.float32
F32R = mybir.dt.float32r
BF16 = mybir.dt.bfloat16
NPBF16 = ml_dtypes.bfloat16

# problem geometry (hardcoded per contest rules)
S = 2048
DIM = 4096
HD = 128
N_HEADS = 32
N_KV = 8
NCORES = 8
HPC = N_HEADS // NCORES       # 4 q heads per core
FEAT = HPC * HD               # 512 per-core attention feature width

SBW = 512                     # phase-1 s-block width
NSB = S // SBW                # 4
KCH = DIM // 128              # 32 contraction chunks
KQ = 8                        # k-chunks per x quarter tile
NXQ = KCH // KQ               # 4
QBW = 512                     # attention q-block width
NQB = S // QBW                # 4
NSC = S // 128                # 16 kv chunks
OBW = 512                     # output-dim block width
NOB = DIM // OBW              # 8
OBS = ["k", 0, "v", 1, 2, 3]  # per-sb output-block order (k first)

_CACHE = {}


def _build():
    nc = bacc.Bacc("TRN2", target_bir_lowering=False, debug=False,
                   num_devices=NCORES)

    xT = nc.dram_tensor("xT", [NSB, NXQ, 128, KQ, SBW], BF16,
                        kind="ExternalInput").ap()
    wqT = nc.dram_tensor("wqT", [HPC, 128, KCH, HD], BF16,
                         kind="ExternalInput").ap()
    wkT = nc.dram_tensor("wkT", [128, KCH, HD], BF16, kind="ExternalInput").ap()
    wvT = nc.dram_tensor("wvT", [128, KCH, HD], BF16, kind="ExternalInput").ap()
    woT = nc.dram_tensor("woT", [HPC, 128, DIM], BF16, kind="ExternalInput").ap()
    cos2 = nc.dram_tensor("cos2", [128, S], F32, kind="ExternalInput").ap()
    sin2 = nc.dram_tensor("sin2", [128, S], F32, kind="ExternalInput").ap()
    sgn = nc.dram_tensor("sgn", [128, 1], F32, kind="ExternalInput").ap()
    onesc = nc.dram_tensor("onesc", [128, 1], BF16, kind="ExternalInput").ap()
    onesr = nc.dram_tensor("onesr", [1, 128], BF16, kind="ExternalInput").ap()
    hm128 = nc.dram_tensor("hm128", [128, 128], BF16, kind="ExternalInput").ap()
    identb = nc.dram_tensor("identb", [128, 128], BF16,
                            kind="ExternalInput").ap()
    out_d = nc.dram_tensor("out", [NQB * HPC, NOB, 128, OBW], BF16,
                           kind="ExternalOutput").ap()

    with tile.TileContext(nc) as tc:
        with (
            tc.tile_pool(name="res", bufs=1) as res,
            tc.tile_pool(name="wo", bufs=1) as wop,
        ):
            kt_t = res.tile([128, S], BF16, tag="kt")
            vt_t = res.tile([128, S], BF16, tag="vt")
            q_all = res.tile([128, HPC, S], BF16, tag="qall")
            v_t = res.tile([128, NSC, HD], BF16, tag="v")
            sgn_t = res.tile([128, 1], F32, tag="sgn")
            onesc_t = res.tile([128, 1], BF16, tag="onesc")
            onesr_t = res.tile([1, 128], BF16, tag="onesr")
            hm_t = res.tile([128, 128], BF16, tag="hm")
            idb_t = res.tile([128, 128], BF16, tag="idb")

            # ---------------- Phase 1: QKV projections + RoPE ----------------
            with (
                tc.tile_pool(name="wq", bufs=1) as wqp,
                tc.tile_pool(name="wkv", bufs=1) as wkvp,
                tc.tile_pool(name="xt", bufs=5) as xtp,
                tc.tile_pool(name="trig", bufs=4) as trigp,
                tc.tile_pool(name="rope", bufs=6) as ropep,
                tc.tile_pool(name="qkvps", bufs=6, space="PSUM") as qkvps,
            ):
                # Startup: spread the critical first-sb loads across BOTH
                # hwdge queues (sync + scalar, ~110GB/s each) in fine slices
                # so the first matmuls start ~6us in instead of ~16us.
                wk_t = wkvp.tile([128, KCH, HD], BF16, tag="wk")
                wv_t = wkvp.tile([128, KCH, HD], BF16, tag="wv")
                wq_hs = [wqp.tile([128, KCH, HD], BF16, tag=f"wq{h}",
                                  name=f"wq{h}")
                         for h in range(HPC)]
                xq_tiles = {}

                def load_xq(sb, q, fine=False):
                    t = xtp.tile([128, KQ, SBW], BF16, tag="xt",
                                 name=f"x{sb}_{q}")
                    if fine:
                        for i in range(4):
                            nc.scalar.dma_start(out=t[:, 2 * i:2 * i + 2],
                                                in_=xT[sb, q, :, 2 * i:2 * i + 2])
                    else:
                        nc.sync.dma_start(out=t[:, 0:4], in_=xT[sb, q, :, 0:4])
                        nc.sync.dma_start(out=t[:, 4:8], in_=xT[sb, q, :, 4:8])
                    xq_tiles[(sb, q)] = t

                # sync:   wk (quartered), x(0,0) fine, wq1, x(0,1), wq3, x...
                # scalar: smalls, wq0 (halved), wv, wq2, wo, per-sb trig
                nc.sync.dma_start(out=wk_t[:, 0:8], in_=wkT[:, 0:8])
                load_xq(0, 0, fine=True)
                nc.scalar.dma_start(out=sgn_t, in_=sgn)
                nc.sync.dma_start(out=wk_t[:, 8:KCH], in_=wkT[:, 8:KCH])
                nc.sync.dma_start(out=wq_hs[0][:, 0:16], in_=wqT[0][:, 0:16])
                nc.scalar.dma_start(out=wq_hs[0][:, 16:KCH],
                                    in_=wqT[0][:, 16:KCH])
                nc.sync.dma_start(out=wq_hs[1], in_=wqT[1])
                nc.scalar.dma_start(out=wv_t, in_=wvT)
                load_xq(0, 1)
                nc.sync.dma_start(out=wq_hs[3], in_=wqT[3])
                nc.scalar.dma_start(out=wq_hs[2], in_=wqT[2])

                nc.scalar.dma_start(out=onesc_t, in_=onesc)
                nc.scalar.dma_start(out=onesr_t, in_=onesr)
                nc.scalar.dma_start(out=hm_t, in_=hm128)
                nc.scalar.dma_start(out=idb_t, in_=identb)
                c0 = trigp.tile([128, SBW], F32, tag="cos")
                nc.scalar.dma_start(out=c0, in_=cos2[:, 0:SBW])
                s0 = trigp.tile([128, SBW], F32, tag="sin")
                nc.scalar.dma_start(out=s0, in_=sin2[:, 0:SBW])
                # wo prefetch (4MB bf16) behind the first trig slices
                wo_hs = []
                for h in range(HPC):
                    wo_h = wop.tile([128, DIM], BF16, tag=f"wo{h}")
                    nc.scalar.dma_start(out=wo_h, in_=woT[h])
                    wo_hs.append(wo_h)

                seq = [(sb, q) for sb in range(NSB) for q in range(NXQ)]
                issued = 2  # (0,0) and (0,1) already issued

                def wsrc(ob, kc):
                    if ob == "k":
                        return wk_t[:, kc, :]
                    if ob == "v":
                        return wv_t[:, kc, :]
                    return wq_hs[ob][:, kc, :]

                for sb in range(NSB):
                    if sb == 0:
                        c_sl, s_sl = c0, s0
                    else:
                        c_sl = trigp.tile([128, SBW], F32, tag="cos")
                        nc.scalar.dma_start(out=c_sl, in_=cos2[:, ts(sb, SBW)])
                        s_sl = trigp.tile([128, SBW], F32, tag="sin")
                        nc.scalar.dma_start(out=s_sl, in_=sin2[:, ts(sb, SBW)])
                    ps = {ob: qkvps.tile([128, SBW], F32, tag="ps",
                                         name=f"ps{sb}_{ob}")
                          for ob in OBS}
                    for q in range(NXQ):
                        # keep 3 quarters of prefetch in flight
                        while issued < len(seq) and issued <= sb * NXQ + q + 3:
                            load_xq(*seq[issued])
                            issued += 1
                        xt = xq_tiles.pop((sb, q))
                        for ob in OBS:
                            for k in range(KQ):
                                kc = q * KQ + k
                                nc.tensor.matmul(ps[ob], wsrc(ob, kc),
                                                 xt[:, k, :],
                                                 start=(kc == 0),
                                                 stop=(kc == KCH - 1))
                    # RoPE / copies in OBS order; rot = swap(x*sin)*sgn + x*cos
                    for ob in OBS:
                        if ob == "v":
                            nc.vector.tensor_copy(vt_t[:, ts(sb, SBW)], ps[ob])
                            continue
                        m1 = ropep.tile([128, SBW], F32, tag="m1")
                        m2 = ropep.tile([128, SBW], F32, tag="m2")
                        w_ = ropep.tile([128, SBW], F32, tag="w")
                        nc.vector.tensor_mul(m1, ps[ob], c_sl)
                        nc.vector.tensor_mul(m2, ps[ob], s_sl)
                        nc.scalar.dma_start(out=w_[0:64], in_=m2[64:128])
                        nc.scalar.dma_start(out=w_[64:128], in_=m2[0:64])
                        dst = (kt_t[:, ts(sb, SBW)] if ob == "k"
                               else q_all[:, ob, ts(sb, SBW)])
                        nc.vector.scalar_tensor_tensor(
                            dst, w_, sgn_t, m1,
                            op0=mybir.AluOpType.mult, op1=mybir.AluOpType.add)

            # ---------------- Phase 2: attention + output projection --------
            with (
                tc.tile_pool(name="exp", bufs=6) as expp,
                tc.tile_pool(name="pair", bufs=3) as pairp,
                tc.tile_pool(name="sum", bufs=3) as sump,
                tc.tile_pool(name="outT", bufs=8) as outTp,
                tc.tile_pool(name="rc", bufs=2) as rcp,
                tc.tile_pool(name="lsb", bufs=2) as lp,
                tc.tile_pool(name="ost", bufs=6) as ostp,
                tc.tile_pool(name="scps", bufs=2, space="PSUM") as scps,
                tc.tile_pool(name="pvps", bufs=2, space="PSUM") as pvps,
                tc.tile_pool(name="normps", bufs=1, space="PSUM") as normps,
                tc.tile_pool(name="prps", bufs=3, space="PSUM") as prps,
            ):
                def transpose_v(sc):
                    tr_ps = scps.tile([128, HD], BF16, tag="sc")
                    nc.tensor.transpose(tr_ps, vt_t[:, ts(sc, 128)], idb_t)
                    nc.vector.tensor_copy(v_t[:, sc, :], tr_ps)

                for sc in range(4):
                    transpose_v(sc)

                def proj_items(pqb, qs):
                    """Yield (emit_fn) items for output-projection slot."""
                    tiles = proj_tiles[pqb]
                    slot = pqb * HPC + qs
                    for ob in range(NOB):
                        p_ps = prps.tile([128, OBW], F32, tag="pr")
                        for h2 in range(HPC):
                            yield lambda ob=ob, h2=h2, p_ps=p_ps: \
                                nc.tensor.matmul(p_ps,
                                                 tiles[h2][:, ts(qs, 128)],
                                                 wo_hs[h2][:, ts(ob, OBW)],
                                                 start=(h2 == 0),
                                                 stop=(h2 == HPC - 1))
                        # PSUM->SBUF copy (alternate vector/scalar), then a
                        # contiguous 128KB tile store (alternate sync/scalar)
                        def fin(ob=ob, p_ps=p_ps):
                            stg = ostp.tile([128, OBW], BF16, tag="ost")
                            if ob % 2 == 0:
                                nc.vector.tensor_copy(stg, p_ps)
                                nc.sync.dma_start(out=out_d[slot, ob], in_=stg)
                            else:
                                nc.scalar.copy(stg, p_ps)
                                nc.scalar.dma_start(out=out_d[slot, ob],
                                                    in_=stg)
                        yield fin

                proj_tiles = {}
                for qb in range(NQB):
                    cur = []
                    for h in range(HPC):
                        nsc = 4 * (qb + 1)
                        items = (list(proj_items(qb - 1, h)) if qb > 0 else [])
                        # transposes for kv chunks needed one q-block ahead
                        tsc = 4 + 4 * qb + h if qb < NQB - 1 else None
                        # hold back a few items to fill the PE gap inside the
                        # normalizer chain (l-matmul -> l copy -> bc-matmul)
                        hold = min(6, len(items))
                        n_inter = len(items) - hold
                        per = -(-n_inter // nsc) if n_inter else 0
                        it = 0
                        sum_t = sump.tile([128, QBW], F32R, tag="sum")
                        pv_ps = pvps.tile([128, QBW], F32, tag="pv")
                        pend_pair = None
                        for sc in range(nsc):
                            t = sc - 4 * qb
                            lo = 128 * t if t >= 0 else 0
                            cols = ds(lo, QBW - lo)
                            s_ps = scps.tile([128, QBW], F32, tag="sc")
                            nc.tensor.matmul(
                                s_ps[:, cols], kt_t[:, ts(sc, 128)],
                                q_all[:, h, ds(qb * QBW + lo, QBW - lo)],
                                start=True, stop=(t < 0))
                            if t >= 0:
                                # causal mask: accumulate -1e5 upper-triangle
                                # into the diagonal 128-col slice (PE-local)
                                nc.tensor.matmul(s_ps[:, ds(lo, 128)], idb_t,
                                                 hm_t, start=False, stop=True)
                            e_t = expp.tile([128, QBW], BF16, tag="exp")
                            nc.scalar.activation(
                                e_t[:, cols], s_ps[:, cols],
                                mybir.ActivationFunctionType.Exp)
                            if t >= 0:
                                if sc == 0:
                                    nc.vector.tensor_copy(sum_t, e_t)
                                else:
                                    nc.vector.tensor_add(
                                        sum_t[:, cols],
                                        sum_t.bitcast(F32)[:, cols],
                                        e_t[:, cols])
                            elif pend_pair is None:
                                pend_pair = e_t
                            else:
                                # merge full chunks pairwise on idle gpsimd,
                                # halving the vector accumulation chain
                                p_t = pairp.tile([128, QBW], BF16, tag="p")
                                nc.gpsimd.tensor_add(p_t, pend_pair, e_t)
                                pend_pair = None
                                if sc == 1:
                                    nc.vector.tensor_copy(sum_t, p_t)
                                else:
                                    nc.vector.tensor_add(
                                        sum_t, sum_t.bitcast(F32), p_t)
                            nc.tensor.matmul(pv_ps[:, cols], v_t[:, sc, :],
                                             e_t[:, cols],
                                             start=(sc == 0),
                                             stop=(sc == nsc - 1))
                            for _ in range(per):
                                if it < n_inter:
                                    items[it]()
                                    it += 1
                        while it < n_inter:
                            items[it]()
                            it += 1
                        # normalizer: l = colsum -> broadcast -> recip -> mul,
                        # in bf16 so the PE stays in fast mode; held-back proj
                        # matmuls + a V transpose fill the cross-engine waits
                        sum_b = sump.tile([128, QBW], BF16, tag="sumb")
                        nc.gpsimd.tensor_copy(sum_b, sum_t.bitcast(F32))
                        l_ps = normps.tile([1, QBW], F32, tag="n")
                        nc.tensor.matmul(l_ps, onesc_t, sum_b,
                                         start=True, stop=True)
                        l_sb = lp.tile([1, QBW], BF16, tag="lsb")
                        nc.vector.tensor_copy(l_sb, l_ps)
                        while it < len(items):
                            items[it]()
                            it += 1
                        if tsc is not None and tsc < NSC:
                            transpose_v(tsc)
                        bc_ps = normps.tile([128, QBW], F32, tag="n")
                        nc.tensor.matmul(bc_ps, onesr_t, l_sb,
                                         start=True, stop=True)
                        rc_t = rcp.tile([128, QBW], F32, tag="rc")
                        nc.vector.reciprocal_approx_fast(out=rc_t, in_=bc_ps)
                        outT_t = outTp.tile([128, QBW], BF16, tag="outT")
                        nc.vector.tensor_mul(outT_t, pv_ps, rc_t)
                        cur.append(outT_t)
                    proj_tiles[qb] = cur
                for qs in range(HPC):
                    for item in proj_items(NQB - 1, qs):
                        item()

    nc.compile()
    return nc


def _host_prep(x, wq, wk, wv, wo, freqs_cos, freqs_sin):
    x = np.asarray(x, np.float32)
    wq = np.asarray(wq, np.float32)
    wk = np.asarray(wk, np.float32)
    wv = np.asarray(wv, np.float32)
    wo = np.asarray(wo, np.float32)
    cos = np.asarray(freqs_cos, np.float32)
    sin = np.asarray(freqs_sin, np.float32)

    scale = 1.0 / np.sqrt(np.float32(HD))
    perm = np.concatenate([np.arange(0, HD, 2), np.arange(1, HD, 2)])
    wq_p = ((wq.reshape(N_HEADS, HD, DIM)[:, perm, :])
            .reshape(DIM, DIM) * scale)
    wk_p = (wk.reshape(N_KV, HD, DIM)[:, perm, :]).reshape(N_KV * HD, DIM)

    # x tiled: xT[sb, q, p, k, s] = x[0, sb*SBW+s, (q*KQ+k)*128+p]
    xs = x.reshape(S, DIM)
    xT_tiled = np.ascontiguousarray(
        xs.reshape(NSB, SBW, NXQ, KQ, 128).transpose(0, 2, 4, 3, 1)
    ).astype(NPBF16)

    def wtile(wmat_rows):  # [128, DIM] -> [128, KCH, 128] bf16
        return np.ascontiguousarray(
            wmat_rows.T.reshape(KCH, 128, wmat_rows.shape[0])
            .transpose(1, 0, 2)).astype(NPBF16)

    cos2 = np.ascontiguousarray(np.concatenate([cos.T, cos.T], 0))
    sin2 = np.ascontiguousarray(np.concatenate([sin.T, sin.T], 0))
    sgnv = np.concatenate([-np.ones((64, 1), np.float32),
                           np.ones((64, 1), np.float32)])
    onesc_a = np.ones((128, 1), np.float32)
    onesr_a = np.ones((1, 128), np.float32)
    hm_a = np.where(np.arange(128)[:, None] > np.arange(128)[None, :],
                    np.float32(-1e5), np.float32(0.0)).astype(NPBF16)
    identb_a = np.eye(128, dtype=np.float32).astype(NPBF16)

    in_maps = []
    for c in range(NCORES):
        wq_c = wq_p[c * FEAT:(c + 1) * FEAT]
        wqT_tiled = np.stack([wtile(wq_c[h * HD:(h + 1) * HD])
                              for h in range(HPC)])
        woc = wo[:, c * FEAT:(c + 1) * FEAT].T  # [FEAT, DIM]
        wo_tiled = np.ascontiguousarray(
            woc.reshape(HPC, 128, DIM)).astype(NPBF16)
        in_maps.append({
            "xT": xT_tiled,
            "wqT": wqT_tiled,
            "wkT": wtile(wk_p[c * HD:(c + 1) * HD]),
            "wvT": wtile(wv[c * HD:(c + 1) * HD]),
            "woT": wo_tiled,
            "cos2": cos2,
            "sin2": sin2,
            "sgn": sgnv,
            "onesc": onesc_a.astype(NPBF16),
            "onesr": onesr_a.astype(NPBF16),
            "hm128": hm_a,
            "identb": identb_a,
        })
    return in_maps


def kernel(x, wq, wk, wv, wo, freqs_cos, freqs_sin, _trace=False):
    if "nc" not in _CACHE:
        _CACHE["nc"] = _build()
    nc = _CACHE["nc"]
    in_maps = _host_prep(x, wq, wk, wv, wo, freqs_cos, freqs_sin)
    res = run_bass_kernel_spmd(nc, in_maps, core_ids=list(range(NCORES)),
                               trace=_trace)
    _CACHE["last_result"] = res
    total = np.zeros((NQB * HPC, NOB, 128, OBW), np.float32)
    for c in range(NCORES):
        total += res.results[c]["out"].astype(np.float32)
    return np.ascontiguousarray(total.transpose(0, 2, 1, 3)).reshape(
        1, S, DIM)


# revision 16
# speedup vs baseline: 1.0822x; 1.0822x over previous
"""TRN2 Bass kernel for GQA attention (nn_Attention_19533511262498).

Tensor-parallel over heads across 8 NeuronCores: core c owns q-heads
[4c, 4c+4) and kv-head c (wq/wk/wv sharded on the head dim, wo on its
input dim). Each core computes a partial [S, DIM] output (bf16); the
host sums the 8 partials in f32.

All matmuls run bf16 x bf16 (both PE operands must share width class;
bf16 LDWEIGHTS is ~97ns and hides fully under 216ns moving-512
matmuls, unlike f32r's 187ns weight loads). End-to-end max-rel error
vs the f32 reference is ~5e-3 (gate 2e-2).

Phase 1 (projections + RoPE): s-blocks of 512, x streamed as k-quarter
tiles; all 6 output blocks (4 q heads, k, v) accumulate concurrently
in 6 PSUM banks so x tiles die quickly. RoPE via the even/odd row
permutation of wq/wk (host side) -> elementwise ops + a half-tile
partition swap (SBUF-SBUF DMA).

Phase 2 (attention + output projection): causal triangle is exploited
at 128-column granularity: diagonal kv-chunks compute only the valid
column suffix (matmul/exp/sum/pv all sliced), with a per-element
[128,128] triangle mask applied to exp values by the vector engine --
no mask matmuls. Softmax normalizer = ones-column reduce matmul ->
broadcast matmul -> vector reciprocal+mul (no gpsimd). The previous
q-block's output projection matmuls are interleaved INTO the score/pv
chunk stream so the PE never waits on the scalar engine's exp chain.
V is transposed on-chip by 16 PE identity transposes interleaved the
same way.
"""

import ml_dtypes
import numpy as np

import concourse.bacc as bacc
import concourse.tile as tile
from concourse import mybir
from concourse.bass import ts, ds
from concourse.bass_utils import run_bass_kernel_spmd

F32 = mybir.dt.float32
F32R = mybir.dt.float32r
BF16 = mybir.dt.bfloat16
NPBF16 = ml_dtypes.bfloat16

# problem geometry (hardcoded per contest rules)
S = 2048
DIM = 4096
HD = 128
N_HEADS = 32
N_KV = 8
NCORES = 8
HPC = N_HEADS // NCORES       # 4 q heads per core
FEAT = HPC * HD               # 512 per-core attention feature width

SBW = 512                     # phase-1 s-block width
NSB = S // SBW                # 4
KCH = DIM // 128              # 32 contraction chunks
KQ = 8                        # k-chunks per x quarter tile
NXQ = KCH // KQ               # 4
QBW = 512                     # attention q-block width
NQB = S // QBW                # 4
NSC = S // 128                # 16 kv chunks
OBW = 512                     # output-dim block width
NOB = DIM // OBW              # 8
OBS = ["k", 0, "v", 1, 2, 3]  # per-sb output-block order (k first)

_CACHE = {}


def _build():
    nc = bacc.Bacc("TRN2", target_bir_lowering=False, debug=False,
                   num_devices=NCORES)

    xT = nc.dram_tensor("xT", [NSB, NXQ, 128, KQ, SBW], BF16,
                        kind="ExternalInput").ap()
    wqT = nc.dram_tensor("wqT", [HPC, 128, KCH, HD], BF16,
                         kind="ExternalInput").ap()
    wkT = nc.dram_tensor("wkT", [128, KCH, HD], BF16, kind="ExternalInput").ap()
    wvT = nc.dram_tensor("wvT", [128, KCH, HD], BF16, kind="ExternalInput").ap()
    woT = nc.dram_tensor("woT", [HPC, 128, DIM], BF16, kind="ExternalInput").ap()
    cos2 = nc.dram_tensor("cos2", [128, S], F32, kind="ExternalInput").ap()
    sin2 = nc.dram_tensor("sin2", [128, S], F32, kind="ExternalInput").ap()
    sgn = nc.dram_tensor("sgn", [128, 1], F32, kind="ExternalInput").ap()
    onesc = nc.dram_tensor("onesc", [128, 1], BF16, kind="ExternalInput").ap()
    onesr = nc.dram_tensor("onesr", [1, 128], BF16, kind="ExternalInput").ap()
    hm128 = nc.dram_tensor("hm128", [128, 128], BF16, kind="ExternalInput").ap()
    identb = nc.dram_tensor("identb", [128, 128], BF16,
                            kind="ExternalInput").ap()
    out_d = nc.dram_tensor("out", [NQB * HPC, NOB, 128, OBW], BF16,
                           kind="ExternalOutput").ap()

    with tile.TileContext(nc) as tc:
        with (
            tc.tile_pool(name="res", bufs=1) as res,
            tc.tile_pool(name="wo", bufs=1) as wop,
        ):
            kt_t = res.tile([128, S], BF16, tag="kt")
            vt_t = res.tile([128, S], BF16, tag="vt")
            q_all = res.tile([128, HPC, S], BF16, tag="qall")
            v_t = res.tile([128, NSC, HD], BF16, tag="v")
            sgn_t = res.tile([128, 1], F32, tag="sgn")
            onesc_t = res.tile([128, 1], BF16, tag="onesc")
            onesr_t = res.tile([1, 128], BF16, tag="onesr")
            hm_t = res.tile([128, 128], BF16, tag="hm")
            idb_t = res.tile([128, 128], BF16, tag="idb")

            # ---------------- Phase 1: QKV projections + RoPE ----------------
            with (
                tc.tile_pool(name="wq", bufs=1) as wqp,
                tc.tile_pool(name="wkv", bufs=1) as wkvp,
                tc.tile_pool(name="xt", bufs=5) as xtp,
                tc.tile_pool(name="trig", bufs=4) as trigp,
                tc.tile_pool(name="rope", bufs=6) as ropep,
                tc.tile_pool(name="qkvps", bufs=6, space="PSUM") as qkvps,
            ):
                # Startup: spread the critical first-sb loads across BOTH
                # hwdge queues (sync + scalar, ~110GB/s each) in fine slices
                # so the first matmuls start ~6us in instead of ~16us.
                wk_t = wkvp.tile([128, KCH, HD], BF16, tag="wk")
                wv_t = wkvp.tile([128, KCH, HD], BF16, tag="wv")
                wq_hs = [wqp.tile([128, KCH, HD], BF16, tag=f"wq{h}",
                                  name=f"wq{h}")
                         for h in range(HPC)]
                xq_tiles = {}

                def load_xq(sb, q, fine=False):
                    t = xtp.tile([128, KQ, SBW], BF16, tag="xt",
                                 name=f"x{sb}_{q}")
                    if fine:
                        for i in range(4):
                            nc.scalar.dma_start(out=t[:, 2 * i:2 * i + 2],
                                                in_=xT[sb, q, :, 2 * i:2 * i + 2])
                    else:
                        nc.sync.dma_start(out=t[:, 0:4], in_=xT[sb, q, :, 0:4])
                        nc.sync.dma_start(out=t[:, 4:8], in_=xT[sb, q, :, 4:8])
                    xq_tiles[(sb, q)] = t

                # sync:   wk (quartered), x(0,0) fine, wq1, x(0,1), wq3, x...
                # scalar: smalls, wq0 (halved), wv, wq2, wo, per-sb trig
                nc.sync.dma_start(out=wk_t[:, 0:8], in_=wkT[:, 0:8])
                load_xq(0, 0, fine=True)
                nc.scalar.dma_start(out=sgn_t, in_=sgn)
                nc.sync.dma_start(out=wk_t[:, 8:KCH], in_=wkT[:, 8:KCH])
                nc.sync.dma_start(out=wq_hs[0][:, 0:16], in_=wqT[0][:, 0:16])
                nc.scalar.dma_start(out=wq_hs[0][:, 16:KCH],
                                    in_=wqT[0][:, 16:KCH])
                nc.sync.dma_start(out=wq_hs[1], in_=wqT[1])
                nc.scalar.dma_start(out=wv_t, in_=wvT)
                load_xq(0, 1)
                nc.sync.dma_start(out=wq_hs[3], in_=wqT[3])
                nc.scalar.dma_start(out=wq_hs[2], in_=wqT[2])

                nc.scalar.dma_start(out=onesc_t, in_=onesc)
                nc.scalar.dma_start(out=onesr_t, in_=onesr)
                nc.scalar.dma_start(out=hm_t, in_=hm128)
                nc.scalar.dma_start(out=idb_t, in_=identb)
                c0 = trigp.tile([128, SBW], F32, tag="cos")
                nc.scalar.dma_start(out=c0, in_=cos2[:, 0:SBW])
                s0 = trigp.tile([128, SBW], F32, tag="sin")
                nc.scalar.dma_start(out=s0, in_=sin2[:, 0:SBW])
                # wo prefetch (4MB bf16) behind the first trig slices
                wo_hs = []
                for h in range(HPC):
                    wo_h = wop.tile([128, DIM], BF16, tag=f"wo{h}")
                    nc.scalar.dma_start(out=wo_h, in_=woT[h])
                    wo_hs.append(wo_h)

                seq = [(sb, q) for sb in range(NSB) for q in range(NXQ)]
                issued = 2  # (0,0) and (0,1) already issued

                def wsrc(ob, kc):
                    if ob == "k":
                        return wk_t[:, kc, :]
                    if ob == "v":
                        return wv_t[:, kc, :]
                    return wq_hs[ob][:, kc, :]

                for sb in range(NSB):
                    if sb == 0:
                        c_sl, s_sl = c0, s0
                    else:
                        c_sl = trigp.tile([128, SBW], F32, tag="cos")
                        nc.scalar.dma_start(out=c_sl, in_=cos2[:, ts(sb, SBW)])
                        s_sl = trigp.tile([128, SBW], F32, tag="sin")
                        nc.scalar.dma_start(out=s_sl, in_=sin2[:, ts(sb, SBW)])
                    ps = {ob: qkvps.tile([128, SBW], F32, tag="ps",
                                         name=f"ps{sb}_{ob}")
                          for ob in OBS}
                    for q in range(NXQ):
                        # keep 3 quarters of prefetch in flight
                        while issued < len(seq) and issued <= sb * NXQ + q + 3:
                            load_xq(*seq[issued])
                            issued += 1
                        xt = xq_tiles.pop((sb, q))
                        for ob in OBS:
                            for k in range(KQ):
                                kc = q * KQ + k
                                nc.tensor.matmul(ps[ob], wsrc(ob, kc),
                                                 xt[:, k, :],
                                                 start=(kc == 0),
                                                 stop=(kc == KCH - 1))
                    # RoPE / copies in OBS order; rot = swap(x*sin)*sgn + x*cos
                    for ob in OBS:
                        if ob == "v":
                            nc.vector.tensor_copy(vt_t[:, ts(sb, SBW)], ps[ob])
                            continue
                        m1 = ropep.tile([128, SBW], F32, tag="m1")
                        m2 = ropep.tile([128, SBW], F32, tag="m2")
                        w_ = ropep.tile([128, SBW], F32, tag="w")
                        nc.vector.tensor_mul(m1, ps[ob], c_sl)
                        nc.vector.tensor_mul(m2, ps[ob], s_sl)
                        nc.scalar.dma_start(out=w_[0:64], in_=m2[64:128])
                        nc.scalar.dma_start(out=w_[64:128], in_=m2[0:64])
                        dst = (kt_t[:, ts(sb, SBW)] if ob == "k"
                               else q_all[:, ob, ts(sb, SBW)])
                        nc.vector.scalar_tensor_tensor(
                            dst, w_, sgn_t, m1,
                            op0=mybir.AluOpType.mult, op1=mybir.AluOpType.add)

            # ---------------- Phase 2: attention + output projection --------
            with (
                tc.tile_pool(name="exp", bufs=6) as expp,
                tc.tile_pool(name="sum", bufs=3) as sump,
                tc.tile_pool(name="outT", bufs=8) as outTp,
                tc.tile_pool(name="rc", bufs=2) as rcp,
                tc.tile_pool(name="lsb", bufs=2) as lp,
                tc.tile_pool(name="ost", bufs=6) as ostp,
                tc.tile_pool(name="scps", bufs=2, space="PSUM") as scps,
                tc.tile_pool(name="pvps", bufs=2, space="PSUM") as pvps,
                tc.tile_pool(name="normps", bufs=1, space="PSUM") as normps,
                tc.tile_pool(name="prps", bufs=3, space="PSUM") as prps,
            ):
                def transpose_v(sc):
                    tr_ps = scps.tile([128, HD], BF16, tag="sc")
                    nc.tensor.transpose(tr_ps, vt_t[:, ts(sc, 128)], idb_t)
                    nc.vector.tensor_copy(v_t[:, sc, :], tr_ps)

                for sc in range(4):
                    transpose_v(sc)

                def proj_items(pqb, qs):
                    """Yield (emit_fn) items for output-projection slot."""
                    tiles = proj_tiles[pqb]
                    slot = pqb * HPC + qs
                    for ob in range(NOB):
                        p_ps = prps.tile([128, OBW], F32, tag="pr")
                        for h2 in range(HPC):
                            yield lambda ob=ob, h2=h2, p_ps=p_ps: \
                                nc.tensor.matmul(p_ps,
                                                 tiles[h2][:, ts(qs, 128)],
                                                 wo_hs[h2][:, ts(ob, OBW)],
                                                 start=(h2 == 0),
                                                 stop=(h2 == HPC - 1))
                        # PSUM->SBUF copy (alternate vector/scalar), then a
                        # contiguous 128KB tile store (alternate sync/scalar)
                        def fin(ob=ob, p_ps=p_ps):
                            stg = ostp.tile([128, OBW], BF16, tag="ost")
                            if ob % 2 == 0:
                                nc.vector.tensor_copy(stg, p_ps)
                                nc.sync.dma_start(out=out_d[slot, ob], in_=stg)
                            else:
                                nc.scalar.copy(stg, p_ps)
                                nc.scalar.dma_start(out=out_d[slot, ob],
                                                    in_=stg)
                        yield fin

                proj_tiles = {}
                for qb in range(NQB):
                    cur = []
                    for h in range(HPC):
                        nsc = 4 * (qb + 1)
                        items = (list(proj_items(qb - 1, h)) if qb > 0 else [])
                        # transposes for kv chunks needed one q-block ahead
                        tsc = 4 + 4 * qb + h if qb < NQB - 1 else None
                        # hold back a few items to fill the PE gap inside the
                        # normalizer chain (l-matmul -> l copy -> bc-matmul)
                        hold = min(6, len(items))
                        n_inter = len(items) - hold
                        per = -(-n_inter // nsc) if n_inter else 0
                        it = 0
                        sum_t = sump.tile([128, QBW], F32R, tag="sum")
                        pv_ps = pvps.tile([128, QBW], F32, tag="pv")
                        for sc in range(nsc):
                            t = sc - 4 * qb
                            lo = 128 * t if t >= 0 else 0
                            cols = ds(lo, QBW - lo)
                            s_ps = scps.tile([128, QBW], F32, tag="sc")
                            nc.tensor.matmul(
                                s_ps[:, cols], kt_t[:, ts(sc, 128)],
                                q_all[:, h, ds(qb * QBW + lo, QBW - lo)],
                                start=True, stop=(t < 0))
                            if t >= 0:
                                # causal mask: accumulate -1e5 upper-triangle
                                # into the diagonal 128-col slice (PE-local)
                                nc.tensor.matmul(s_ps[:, ds(lo, 128)], idb_t,
                                                 hm_t, start=False, stop=True)
                            e_t = expp.tile([128, QBW], BF16, tag="exp")
                            nc.scalar.activation(
                                e_t[:, cols], s_ps[:, cols],
                                mybir.ActivationFunctionType.Exp)
                            if sc == 0:
                                nc.vector.tensor_copy(sum_t, e_t)
                            else:
                                nc.vector.tensor_add(
                                    sum_t[:, cols],
                                    sum_t.bitcast(F32)[:, cols], e_t[:, cols])
                            nc.tensor.matmul(pv_ps[:, cols], v_t[:, sc, :],
                                             e_t[:, cols],
                                             start=(sc == 0),
                                             stop=(sc == nsc - 1))
                            for _ in range(per):
                                if it < n_inter:
                                    items[it]()
                                    it += 1
                        while it < n_inter:
                            items[it]()
                            it += 1
                        # normalizer: l = colsum -> broadcast -> recip -> mul,
                        # in bf16 so the PE stays in fast mode; held-back proj
                        # matmuls + a V transpose fill the cross-engine waits
                        sum_b = sump.tile([128, QBW], BF16, tag="sumb")
                        nc.vector.tensor_copy(sum_b, sum_t.bitcast(F32))
                        l_ps = normps.tile([1, QBW], F32, tag="n")
                        nc.tensor.matmul(l_ps, onesc_t, sum_b,
                                         start=True, stop=True)
                        l_sb = lp.tile([1, QBW], BF16, tag="lsb")
                        nc.vector.tensor_copy(l_sb, l_ps)
                        while it < len(items):
                            items[it]()
                            it += 1
                        if tsc is not None and tsc < NSC:
                            transpose_v(tsc)
                        bc_ps = normps.tile([128, QBW], F32, tag="n")
                        nc.tensor.matmul(bc_ps, onesr_t, l_sb,
                                         start=True, stop=True)
                        rc_t = rcp.tile([128, QBW], F32, tag="rc")
                        nc.vector.reciprocal_approx_fast(out=rc_t, in_=bc_ps)
                        outT_t = outTp.tile([128, QBW], BF16, tag="outT")
                        nc.vector.tensor_mul(outT_t, pv_ps, rc_t)
                        cur.append(outT_t)
                    proj_tiles[qb] = cur
                for qs in range(HPC):
                    for item in proj_items(NQB - 1, qs):
                        item()

    nc.compile()
    return nc


def _host_prep(x, wq, wk, wv, wo, freqs_cos, freqs_sin):
    x = np.asarray(x, np.float32)
    wq = np.asarray(wq, np.float32)
    wk = np.asarray(wk, np.float32)
    wv = np.asarray(wv, np.float32)
    wo = np.asarray(wo, np.float32)
    cos = np.asarray(freqs_cos, np.float32)
    sin = np.asarray(freqs_sin, np.float32)

    scale = 1.0 / np.sqrt(np.float32(HD))
    perm = np.concatenate([np.arange(0, HD, 2), np.arange(1, HD, 2)])
    wq_p = ((wq.reshape(N_HEADS, HD, DIM)[:, perm, :])
            .reshape(DIM, DIM) * scale)
    wk_p = (wk.reshape(N_KV, HD, DIM)[:, perm, :]).reshape(N_KV * HD, DIM)

    # x tiled: xT[sb, q, p, k, s] = x[0, sb*SBW+s, (q*KQ+k)*128+p]
    xs = x.reshape(S, DIM)
    xT_tiled = np.ascontiguousarray(
        xs.reshape(NSB, SBW, NXQ, KQ, 128).transpose(0, 2, 4, 3, 1)
    ).astype(NPBF16)

    def wtile(wmat_rows):  # [128, DIM] -> [128, KCH, 128] bf16
        return np.ascontiguousarray(
            wmat_rows.T.reshape(KCH, 128, wmat_rows.shape[0])
            .transpose(1, 0, 2)).astype(NPBF16)

    cos2 = np.ascontiguousarray(np.concatenate([cos.T, cos.T], 0))
    sin2 = np.ascontiguousarray(np.concatenate([sin.T, sin.T], 0))
    sgnv = np.concatenate([-np.ones((64, 1), np.float32),
                           np.ones((64, 1), np.float32)])
    onesc_a = np.ones((128, 1), np.float32)
    onesr_a = np.ones((1, 128), np.float32)
    hm_a = np.where(np.arange(128)[:, None] > np.arange(128)[None, :],
                    np.float32(-1e5), np.float32(0.0)).astype(NPBF16)
    identb_a = np.eye(128, dtype=np.float32).astype(NPBF16)

    in_maps = []
    for c in range(NCORES):
        wq_c = wq_p[c * FEAT:(c + 1) * FEAT]
        wqT_tiled = np.stack([wtile(wq_c[h * HD:(h + 1) * HD])
                              for h in range(HPC)])
        woc = wo[:, c * FEAT:(c + 1) * FEAT].T  # [FEAT, DIM]
        wo_tiled = np.ascontiguousarray(
            woc.reshape(HPC, 128, DIM)).astype(NPBF16)
        in_maps.append({
            "xT": xT_tiled,
            "wqT": wqT_tiled,
            "wkT": wtile(wk_p[c * HD:(c + 1) * HD]),
            "wvT": wtile(wv[c * HD:(c + 1) * HD]),
            "woT": wo_tiled,
            "cos2": cos2,
            "sin2": sin2,
            "sgn": sgnv,
            "onesc": onesc_a.astype(NPBF16),
            "onesr": onesr_a.astype(NPBF16),
            "hm128": hm_a,
            "identb": identb_a,
        })
    return in_maps


def kernel(x, wq, wk, wv, wo, freqs_cos, freqs_sin, _trace=False):
    if "nc" not in _CACHE:
        _CACHE["nc"] = _build()
    nc = _CACHE["nc"]
    in_maps = _host_prep(x, wq, wk, wv, wo, freqs_cos, freqs_sin)
    res = run_bass_kernel_spmd(nc, in_maps, core_ids=list(range(NCORES)),
                               trace=_trace)
    _CACHE["last_result"] = res
    total = np.zeros((NQB * HPC, NOB, 128, OBW), np.float32)
    for c in range(NCORES):
        total += res.results[c]["out"].astype(np.float32)
    return np.ascontiguousarray(total.transpose(0, 2, 1, 3)).reshape(
        1, S, DIM)
